# revision 73
# baseline (speedup 1.0000x reference)
"""AgglutinativeAttention Trainium2 kernel.

Full inputs in, full output out. Sharding: 8 cores = (batch b in 0..3) x
(head-group g in 0..1). Each core computes, for its batch b and its 8 heads:
  qT/kT = (x @ W{q,k}[:, gF:(g+1)F])^T   [512 feat, 1024 tok]
  v     =  x @ Wv[:, gF:(g+1)F]          [1024 tok, 512 feat] (+ones col/head)
  per head: sT = scores transposed [j, i] via fp8 DoubleRow matmuls (dithered
  dual quantization, see the qT/kT comment), morpho verb bias via an ebT
  elementwise factor + per-partition activation bias (col bias),
  pT = exp(scale*sT + cb) * ebT, oT = v_aug^T @ pT with a ones row giving the
  softmax denominator, divide, then partial z = o @ Wo[gF:(g+1)F, :].
  Host sums the two per-batch partials + bo.

x and the q/k/v weights travel as fp8 hi+residual pairs (split-fp8
DoubleRow projections at 0.75x the bf16 PE cost), wo and z as bf16. DMA
order is arranged so the first v-proj matmul is gated on a few hundred KB,
not 7MB; warm-up matmuls burn the PE p-state ramp during the initial DMA
wait.
"""

import numpy as np
import ml_dtypes
from contextlib import ExitStack

import concourse.bass as bass
import concourse.mybir as mybir
import concourse.tile as tile
from concourse import bacc
from concourse.bass_utils import run_bass_kernel_spmd

B, S, H = 4, 1024, 1024
NH, HD = 16, 64
G = 2                 # head groups (tensor-parallel factor per batch)
F = H // G            # 512 features per core
HPC = NH // G         # 8 heads per core
SCALE = 1.0 / np.sqrt(HD)
VERB_BIAS, ROOT_BIAS, SUFFIX_BIAS = 2.0, 1.5, 1.2
BIG = np.float32(1e9)

f32 = mybir.dt.float32
f32r = mybir.dt.float32r
bf16 = mybir.dt.bfloat16
i32 = mybir.dt.int32

P = 128
KC = H // P           # 8 contraction chunks for projections
TC = S // P           # 8 token chunks of 128
IC = S // 512         # 2 chunks of 512 (matmul free dim)
FC = F // P           # 4 feature chunks per core

_COMPILED = None


def _build():
    nc = bacc.Bacc("TRN2", target_bir_lowering=False, debug=False, num_devices=8)

    fp8d = mybir.dt.float8e4
    x_d = nc.dram_tensor("x", [H, S], fp8d, kind="ExternalInput").ap()
    xr_d = nc.dram_tensor("xr", [H, S], fp8d, kind="ExternalInput").ap()
    wq_d = nc.dram_tensor("wq", [H, F], fp8d, kind="ExternalInput").ap()
    wqr_d = nc.dram_tensor("wqr", [H, F], fp8d, kind="ExternalInput").ap()
    wk_d = nc.dram_tensor("wk", [H, F], fp8d, kind="ExternalInput").ap()
    wkr_d = nc.dram_tensor("wkr", [H, F], fp8d, kind="ExternalInput").ap()
    wv_d = nc.dram_tensor("wv", [H, F], fp8d, kind="ExternalInput").ap()
    wvr_d = nc.dram_tensor("wvr", [H, F], fp8d, kind="ExternalInput").ap()
    wo_d = nc.dram_tensor("wo", [F, H], bf16, kind="ExternalInput").ap()
    bqs_d = nc.dram_tensor("bqs", [F], f32, kind="ExternalInput").ap()
    bk_d = nc.dram_tensor("bk", [F], f32, kind="ExternalInput").ap()
    bv_d = nc.dram_tensor("bv", [F], f32, kind="ExternalInput").ap()
    nearf_d = nc.dram_tensor("nearf", [S], f32, kind="ExternalInput").ap()
    cb_d = nc.dram_tensor("cb", [S], f32, kind="ExternalInput").ap()
    z_d = nc.dram_tensor("z", [S, H], bf16, kind="ExternalOutput").ap()

    with tile.TileContext(nc) as tc, ExitStack() as ctx:
        const = ctx.enter_context(tc.tile_pool(name="const", bufs=1))
        big = ctx.enter_context(tc.tile_pool(name="big", bufs=1))
        ppool = ctx.enter_context(tc.tile_pool(name="ppool", bufs=6))
        rlpool = ctx.enter_context(tc.tile_pool(name="rlpool", bufs=4))
        osbpool = ctx.enter_context(tc.tile_pool(name="osbpool", bufs=4))
        zpool = ctx.enter_context(tc.tile_pool(name="zpool", bufs=6))
        ps_q = ctx.enter_context(tc.tile_pool(name="ps_q", bufs=2, space="PSUM"))
        ps_s = ctx.enter_context(tc.tile_pool(name="ps_s", bufs=2, space="PSUM"))
        ps_o = ctx.enter_context(tc.tile_pool(name="ps_o", bufs=1, space="PSUM"))

        # ---- small constants via SWDGE (gpsimd) — keeps HWDGE free for the
        # wv/x stream that gates the first matmul. Only the two needed early
        # (bv for the first v eviction, nearf for the ohst chain) go first;
        # the rest are emitted after the ohst loop so their transfers don't
        # sit ahead of the wv/x stream on the DMA engines ----
        near_row = const.tile([1, S], f32, tag="near_row")
        bv_row = const.tile([1, F], f32, tag="bv_row")
        nc.gpsimd.dma_start(bv_row[:], bv_d[None, :])
        nc.gpsimd.dma_start(near_row[:], nearf_d[None, :])
        cb_sb = const.tile([P, TC], f32, tag="cb_sb")
        bq_sb = const.tile([P, FC], f32, tag="bq_sb")
        bk_sb = const.tile([P, FC], f32, tag="bk_sb")

        # dithered bias copies for the ACT-side slot-1 evictions:
        # ACT computes Copy(ps*(1-h) + bq*(1-h)) == (ps + bq)*(1-h)
        bq_a = const.tile([P, FC], f32, tag="bq_a")
        bk_a = const.tile([P, FC], f32, tag="bk_a")
        bq_b = const.tile([P, FC], f32, tag="bq_b")
        bk_b = const.tile([P, FC], f32, tag="bk_b")

        iota_i = const.tile([P, KC], i32, tag="iota_i")
        nc.gpsimd.iota(iota_i[:], pattern=[[P, KC]], base=0, channel_multiplier=1)
        iota_f = const.tile([P, KC], f32, tag="iota_f")
        nc.gpsimd.tensor_copy(iota_f[:], iota_i[:])

        near_bc = const.tile([P, S], f32, tag="near_bc")
        nc.gpsimd.partition_broadcast(near_bc[:], near_row[:])
        bv_bc = const.tile([P, F], f32, tag="bv_bc")
        nc.gpsimd.partition_broadcast(bv_bc[:], bv_row[:])

        # q/k live as fp8e4 PAIRS: slot 0 holds fp8(v*(1+2^-4)), slot 1 holds
        # fp8(v*(1-2^-4)) — a half-quantization-cell dither. The DoubleRow
        # score matmul (0.5 cycles/row, half the bf16 PE cost) sums both
        # slots, averaging two anti-phased quantization errors (~2x less fp8
        # noise); the exact deterministic scale (a^2+b^2) plus 1/sqrt(d)
        # folds into the exp's scale operand. q itself is NOT pre-scaled so
        # fp8 sees healthy magnitudes.
        fp8 = mybir.dt.float8e4
        DIT = 1.0 / 16.0
        DSC = (1.0 + DIT) ** 2 + (1.0 - DIT) ** 2
        qT = big.tile([P, FC, 2, S], fp8, tag="qT")
        kT = big.tile([P, FC, 2, S], fp8, tag="kT")
        v_sb = big.tile([P, TC, HPC, 65], bf16, tag="v_sb")
        ones64 = const.tile([P, TC * HPC], f32, tag="ones64")
        nc.vector.memset(ones64[:], 1.0)
        nc.vector.tensor_copy(
            v_sb[:, :, :, 64:65],
            ones64.rearrange("p (a b one) -> p a b one", a=TC, b=HPC, one=1),
        )

        # warm-up matmuls on resident constants: the tensor engine p-state
        # ramps only under continuous execution (LOW->MID->full over ~3us),
        # so burn the ramp on throwaway [128x64 @ 128x32] products while the
        # first wv/x DMAs are still in flight — real matmuls then start at
        # full clock
        wps = ps_q.tile([P, 512], f32, tag="ps_proj", name="warmup")
        for w in range(28):
            nc.tensor.matmul(
                wps[0:64, 0:32],
                ones64[:, 0:64],
                ones64[:, 0:32],
                start=True, stop=True,
            )

        # verb factor (transposed): ebT[p, jc, i] = exp(2 * (jc*128+p == nearest[i]))
        # ohst built on gpsimd (idle during the projection phase) so DVE keeps
        # its slots for the v/q/k evictions; exp stays on ACT (also loads the
        # exp table early, during the quiet phase)
        ebT = big.tile([P, TC, S], bf16, tag="ebT")
        ohstage = ctx.enter_context(tc.tile_pool(name="ohstage", bufs=2))
        for jc in range(TC):
            ohst = ohstage.tile([P, S], f32, tag="ohst")
            nc.gpsimd.tensor_scalar(
                ohst[:], near_bc[:], iota_f[:, jc : jc + 1], 2.0,
                mybir.AluOpType.is_equal, mybir.AluOpType.mult,
            )
            nc.scalar.activation(
                ebT[:, jc, :], ohst[:], mybir.ActivationFunctionType.Exp
            )

        projpool = ctx.enter_context(tc.tile_pool(name="projpool", bufs=1))
        wq_sb = projpool.tile([P, KC, F], fp8, tag="wq_sb")
        wqr_sb = projpool.tile([P, KC, F], fp8, tag="wqr_sb")
        wk_sb = projpool.tile([P, KC, F], fp8, tag="wk_sb")
        wkr_sb = projpool.tile([P, KC, F], fp8, tag="wkr_sb")
        xTh = []
        xTr = []
        for i in range(IC):
            xthalf = projpool.tile([P, KC, 512], fp8, tag=f"xT{i}", name=f"xT{i}")
            xTh.append(xthalf)
            xrhalf = projpool.tile([P, KC, 512], fp8, tag=f"xR{i}", name=f"xR{i}")
            xTr.append(xrhalf)

        attn2 = ctx.enter_context(tc.tile_pool(name="attn2", bufs=1))
        oT = attn2.tile([P, FC, S], bf16, tag="oT")
        wo_sb = attn2.tile([P, FC, H], bf16, tag="wo_sb")

        # ---- x arrives host-pre-transposed as fp8 hi+residual; projections
        # run as split-fp8 DoubleRow kc-pair chains: x8*w8 + xr*w8 + x8*wr
        # (the dropped xr*wr term is ~quantization-squared). Weights are
        # host-pre-scaled by 32 into fp8's normal range; 1/32 folds into the
        # q/k eviction scales and (for v) into host-side Wo/32 + bv*32 ----
        with tc.tile_pool(name="wkvpool", bufs=1) as wkvpool:
            wv_sb = wkvpool.tile([P, KC, F], fp8, tag="wv_sb")
            wvr_sb = wkvpool.tile([P, KC, F], fp8, tag="wvr_sb")

            def emit_vproj(tci_range):
                for tci in tci_range:
                    ps = ps_q.tile([P, 512], f32, tag="ps_proj")
                    half = tci // 4
                    tsl = slice((tci % 4) * P, (tci % 4 + 1) * P)
                    k = 0
                    for kc2 in range(0, KC, 2):
                        for xs, ws in (
                            (xTh[half], wv_sb),
                            (xTr[half], wv_sb),
                            (xTh[half], wvr_sb),
                        ):
                            nc.tensor.matmul(
                                ps[:],
                                xs[:, kc2 : kc2 + 2, tsl],
                                ws[:, kc2 : kc2 + 2, :],
                                start=(k == 0), stop=(k == 11),
                                perf_mode=mybir.MatmulPerfMode.DoubleRow,
                            )
                            k += 1
                    nc.vector.tensor_tensor(
                        v_sb[:, tci, :, 0:64],
                        ps.rearrange("p (h d) -> p h d", d=64),
                        bv_bc.rearrange("p (h d) -> p h d", d=64),
                        mybir.AluOpType.add,
                    )

            # DMA chain ordered by first consumption: the first v-proj matmuls
            # need only wv[kc0-3] + x[kc0-3, tok0-255]; everything else streams
            # behind while the PE is already busy
            nc.sync.dma_start(
                wv_sb[:, 0:4, :],
                wv_d[0 : 4 * P, :].rearrange("(kc p) f -> p kc f", p=P),
            )
            nc.sync.dma_start(
                xTh[0][:, 0:4, 0:512],
                x_d[0 : 4 * P, 0:512].rearrange("(kc p) s -> p kc s", p=P),
            )
            nc.sync.dma_start(
                wvr_sb[:, 0:4, :],
                wvr_d[0 : 4 * P, :].rearrange("(kc p) f -> p kc f", p=P),
            )
            nc.sync.dma_start(
                xTr[0][:, 0:4, 0:512],
                xr_d[0 : 4 * P, 0:512].rearrange("(kc p) s -> p kc s", p=P),
            )
            nc.sync.dma_start(
                wv_sb[:, 4:8, :],
                wv_d[4 * P :, :].rearrange("(kc p) f -> p kc f", p=P),
            )
            nc.sync.dma_start(
                xTh[0][:, 4:8, 0:512],
                x_d[4 * P :, 0:512].rearrange("(kc p) s -> p kc s", p=P),
            )
            nc.sync.dma_start(
                wvr_sb[:, 4:8, :],
                wvr_d[4 * P :, :].rearrange("(kc p) f -> p kc f", p=P),
            )
            nc.sync.dma_start(
                xTr[0][:, 4:8, 0:512],
                xr_d[4 * P :, 0:512].rearrange("(kc p) s -> p kc s", p=P),
            )
            nc.sync.dma_start(
                xTh[1][:], x_d[:, 512:1024].rearrange("(kc p) s -> p kc s", p=P)
            )
            nc.sync.dma_start(
                xTr[1][:], xr_d[:, 512:1024].rearrange("(kc p) s -> p kc s", p=P)
            )
            nc.sync.dma_start(wq_sb[:], wq_d.rearrange("(kc p) f -> p kc f", p=P))
            nc.sync.dma_start(wqr_sb[:], wqr_d.rearrange("(kc p) f -> p kc f", p=P))
            nc.sync.dma_start(wk_sb[:], wk_d.rearrange("(kc p) f -> p kc f", p=P))
            nc.sync.dma_start(wkr_sb[:], wkr_d.rearrange("(kc p) f -> p kc f", p=P))
            nc.sync.dma_start(wo_sb[:], wo_d.rearrange("(fc p) o -> p fc o", p=P))
            # remaining small constants, needed from the q/k evictions and
            # first exp (~16us in) — HWDGE at the stream tail, not SWDGE,
            # whose descriptor generation costs Pool engine time and would
            # push the ohst chain back
            nc.sync.dma_start(cb_sb[:], cb_d.rearrange("(jc p) -> p jc", p=P))
            nc.sync.dma_start(bq_sb[:], bqs_d.rearrange("(fc p) -> p fc", p=P))
            nc.sync.dma_start(bk_sb[:], bk_d.rearrange("(fc p) -> p fc", p=P))
            nc.gpsimd.tensor_scalar(
                bq_a[:], bq_sb[:], 1.0 + DIT, None, mybir.AluOpType.mult
            )
            nc.gpsimd.tensor_scalar(
                bk_a[:], bk_sb[:], 1.0 + DIT, None, mybir.AluOpType.mult
            )
            nc.gpsimd.tensor_scalar(
                bq_b[:], bq_sb[:], 1.0 - DIT, None, mybir.AluOpType.mult
            )
            nc.gpsimd.tensor_scalar(
                bk_b[:], bk_sb[:], 1.0 - DIT, None, mybir.AluOpType.mult
            )
            emit_vproj(range(0, 8))

        # ---- attention interleaved with q projection, per head pair.
        # Both oc halves of a tci land in one zt tile -> one DMA per z row
        # block (halves the HWDGE/sem slots; the tail is store-latency bound)
        zts = {}

        def emit_oproj(tiles, tail=False):
            for n, (tci, oc) in enumerate(tiles):
                if tail and n % 2 == 1:
                    pszw = ps_s.tile([P, 1024], f32, tag="pssb", name="pszw")
                    psz = pszw[:, 0:512]
                else:
                    psz = ps_q.tile([P, 512], f32, tag="ps_proj")
                for fc in range(FC):
                    nc.tensor.matmul(
                        psz[:],
                        oT[:, fc, tci * P : (tci + 1) * P],
                        wo_sb[:, fc, oc * 512 : (oc + 1) * 512],
                        start=(fc == 0), stop=(fc == FC - 1),
                    )
                if tci not in zts:
                    zts[tci] = zpool.tile([P, H], bf16, tag="zt", name="zt")
                zt = zts[tci]
                # gpsimd cannot read PSUM on real HW — evictions go DVE/ACT
                if tail:
                    nc.scalar.copy(zt[:, oc * 512 : (oc + 1) * 512], psz[:])
                else:
                    nc.vector.tensor_copy(zt[:, oc * 512 : (oc + 1) * 512], psz[:])
                # last two row blocks ship per-oc half-stores so the final
                # DMA after the last eviction is half-sized
                split_store = tail and tci >= 6
                if split_store:
                    nc.sync.dma_start(
                        z_d[tci * P : (tci + 1) * P, oc * 512 : (oc + 1) * 512],
                        zt[:, oc * 512 : (oc + 1) * 512],
                    )
                    if oc == IC - 1:
                        del zts[tci]
                elif oc == IC - 1:
                    nc.sync.dma_start(z_d[tci * P : (tci + 1) * P, :], zt[:])
                    del zts[tci]

        def flush_divisions(pending, eng=None, eng_alt=False):
            for n, (ic_, fc4_, side_, osb_, rlb_) in enumerate(pending):
                hb = side_ * 64
                ssl = slice(side_ * 512, (side_ + 1) * 512)
                e = (nc.vector if n % 2 else nc.gpsimd) if eng_alt else (eng or nc.gpsimd)
                e.tensor_tensor(
                    oT[hb : hb + 64, fc4_, ic_ * 512 : (ic_ + 1) * 512],
                    osb_[0:64, ssl], rlb_[0:64, ssl],
                    mybir.AluOpType.mult,
                )
            pending.clear()

        def emit_qproj(fc, icq):
            ps = ps_q.tile([P, 512], f32, tag="ps_proj")
            fsl = slice(fc * P, (fc + 1) * P)
            k = 0
            for kc2 in range(0, KC, 2):
                for ws, xs in (
                    (wq_sb, xTh[icq]),
                    (wq_sb, xTr[icq]),
                    (wqr_sb, xTh[icq]),
                ):
                    nc.tensor.matmul(
                        ps[:],
                        ws[:, kc2 : kc2 + 2, fsl],
                        xs[:, kc2 : kc2 + 2, :],
                        start=(k == 0), stop=(k == 11),
                        perf_mode=mybir.MatmulPerfMode.DoubleRow,
                    )
                    k += 1
            # psum holds 32*q (host-scaled weights); (1 +- DIT)/32 restores
            # scale and applies the dither in one tensor_scalar each
            nc.vector.tensor_scalar(
                qT[:, fc, 0, icq * 512 : (icq + 1) * 512], ps[:],
                (1.0 + DIT) / 32.0, bq_a[:, fc : fc + 1],
                mybir.AluOpType.mult, mybir.AluOpType.add,
            )
            nc.vector.tensor_scalar(
                qT[:, fc, 1, icq * 512 : (icq + 1) * 512], ps[:],
                (1.0 - DIT) / 32.0, bq_b[:, fc : fc + 1],
                mybir.AluOpType.mult, mybir.AluOpType.add,
            )

        def emit_kproj(fc, ick):
            ps = ps_q.tile([P, 512], f32, tag="ps_proj")
            fsl = slice(fc * P, (fc + 1) * P)
            k = 0
            for kc2 in range(0, KC, 2):
                for ws, xs in (
                    (wk_sb, xTh[ick]),
                    (wk_sb, xTr[ick]),
                    (wkr_sb, xTh[ick]),
                ):
                    nc.tensor.matmul(
                        ps[:],
                        ws[:, kc2 : kc2 + 2, fsl],
                        xs[:, kc2 : kc2 + 2, :],
                        start=(k == 0), stop=(k == 11),
                        perf_mode=mybir.MatmulPerfMode.DoubleRow,
                    )
                    k += 1
            nc.vector.tensor_scalar(
                kT[:, fc, 0, ick * 512 : (ick + 1) * 512], ps[:],
                (1.0 + DIT) / 32.0, bk_a[:, fc : fc + 1],
                mybir.AluOpType.mult, mybir.AluOpType.add,
            )
            nc.scalar.activation(
                kT[:, fc, 1, ick * 512 : (ick + 1) * 512], ps[:],
                mybir.ActivationFunctionType.Identity,
                bias=bk_b[:, fc : fc + 1], scale=(1.0 - DIT) / 32.0,
            )

        pending = []
        oproj_ic0 = [(tci, oc) for tci in range(4) for oc in range(IC)]
        for fc4 in range(FC):
            if fc4 == 0:
                emit_qproj(0, 0)
                emit_kproj(0, 0)
            for ic in range(IC):
                # heads of the pair interleaved: PE alternates A/B matmuls
                # while ACT/DVE process the other head's exp / verb multiply
                last_group = (fc4 == FC - 1 and ic == IC - 1)
                # one [65, 1024] psum tile holds both heads' attnv output —
                # a single recip / broadcast / eviction per group instead of
                # two halves the group-end instruction count on DVE and Pool
                pso = ps_o.tile([65, 1024], f32, tag="pso", name="pso")
                for jc in range(TC):
                    # both heads' scores into one 2-bank psum tile so a single
                    # [128,1024] exp (and verb multiply) covers the pair —
                    # halves the instruction count and PSUM-access overhead on
                    # the ACT-critical path
                    pssb = ps_s.tile([P, 1024], f32, tag="pssb")
                    for side in range(2):
                        hb = side * 64
                        nc.tensor.matmul(
                            pssb[:, side * 512 : (side + 1) * 512],
                            kT[hb : hb + 64, fc4, 0:2, jc * P : (jc + 1) * P],
                            qT[hb : hb + 64, fc4, 0:2, ic * 512 : (ic + 1) * 512],
                            start=True, stop=True,
                            perf_mode=mybir.MatmulPerfMode.DoubleRow,
                        )
                    pTb = ppool.tile([P, 1024], bf16, tag="pTb")
                    nc.scalar.activation(
                        pTb[:], pssb[:], mybir.ActivationFunctionType.Exp,
                        bias=cb_sb[:, jc : jc + 1], scale=SCALE / DSC,
                    )
                    ebsl = ebT[:, jc, ic * 512 : (ic + 1) * 512]
                    nc.vector.tensor_tensor(
                        pTb.rearrange("p (two n) -> p two n", two=2),
                        pTb.rearrange("p (two n) -> p two n", two=2),
                        ebsl[:, None, :].to_broadcast((P, 2, 512)),
                        mybir.AluOpType.mult,
                    )
                    if ic == 0 and jc == 1:
                        # the second-half k projection rides inside the first
                        # group; q(fc,1) is emitted at the i0->i1 boundary
                        # instead, covering the pso-eviction burst there
                        emit_kproj(fc4, 1)
                    if ic == 1 and fc4 < FC - 1:
                        # next head-pair's first-half projections fill the
                        # otherwise-bare i1 groups (needed a full group later)
                        if jc == 2:
                            emit_qproj(fc4 + 1, 0)
                        if jc == 5:
                            emit_kproj(fc4 + 1, 0)
                    if last_group and jc >= 4 and oproj_ic0:
                        # (f3,i0)'s divisions are flushed at jc0 below and run
                        # on the backlogged Pool/DVE ~2us later — fills start
                        # at jc4 so the in-order PE stream never blocks on them
                        emit_oproj(oproj_ic0[:1])
                        del oproj_ic0[:1]
                    if last_group and jc == 0:
                        flush_divisions(pending, eng_alt=True)
                    for side in range(2):
                        h = 2 * fc4 + side
                        nc.tensor.matmul(
                            pso[:, side * 512 : (side + 1) * 512],
                            v_sb[:, jc, h, 0:65],
                            pTb[:, side * 512 : (side + 1) * 512],
                            start=(jc == 0), stop=(jc == TC - 1),
                        )
                if ic == 0:
                    # boundary chain first: keeps the PE busy across the
                    # i0->i1 group switch while DVE/Pool drain the evictions
                    emit_qproj(fc4, 1)
                # previous group's divisions (their broadcasts completed while
                # this group was streaming) — keeps the division stream
                # stall-free
                flush_divisions(pending, eng_alt=True)
                rlrow = rlpool.tile([1, 1024], f32, tag="rlrow")
                nc.vector.reciprocal(rlrow[:], pso[64:65, :])
                osb = None
                if not last_group:
                    osb = osbpool.tile([65, 1024], f32, tag="osb", name="osb")
                    nc.vector.tensor_copy(osb[:], pso[:])
                rlb = rlpool.tile([64, 1024], f32, tag="rlb")
                nc.gpsimd.partition_broadcast(rlb[:], rlrow[:])
                for side in range(2):
                    ssl = slice(side * 512, (side + 1) * 512)
                    if last_group:
                        # shortest possible chain before the trailing o_proj:
                        # divide straight from psum, no eviction, no deferral
                        hb = side * 64
                        nc.vector.tensor_tensor(
                            oT[hb : hb + 64, fc4, ic * 512 : (ic + 1) * 512],
                            pso[0:64, ssl], rlb[:, ssl],
                            mybir.AluOpType.mult,
                        )
                    else:
                        pending.append((ic, fc4, side, osb, rlb))
        flush_divisions(pending, eng=nc.vector)
        if oproj_ic0:
            emit_oproj(oproj_ic0)
        emit_oproj([(tci, oc) for tci in range(4, 8) for oc in range(IC)], tail=True)

    nc.compile()
    return nc


def _get_compiled():
    global _COMPILED
    if _COMPILED is None:
        _COMPILED = _build()
    return _COMPILED


def _host_morpho(morpho_types):
    """nearest-verb index per (b, i) (-1 if batch has no verb) and col bias."""
    mt = np.asarray(morpho_types)
    pos = np.arange(S)
    dist = np.abs(pos[:, None] - pos[None, :]).astype(np.float32)
    nearest = np.empty((B, S), np.float32)
    for b in range(B):
        is_verb = mt[b] == 2
        if not is_verb.any():
            nearest[b] = -1.0
            continue
        dm = np.where(is_verb[None, :], dist, BIG)
        nearest[b] = np.argmin(dm, axis=-1).astype(np.float32)
    cb = (
        np.float32(ROOT_BIAS * 0.5) * (mt == 0)
        + np.float32(SUFFIX_BIAS * 0.3) * (mt == 1)
    ).astype(np.float32)
    return nearest, cb


def _fp8_split(a):
    f8 = ml_dtypes.float8_e4m3
    hi = np.ascontiguousarray(a.astype(f8))
    res = np.ascontiguousarray((a - hi.astype(np.float32)).astype(f8))
    return hi, res


def build_in_maps(hidden_states, morpho_types, Wq, bq, Wk, bk, Wv, bv, Wo, bo):
    # weights are pre-scaled by 32 into fp8's normal range (their raw 0.02
    # scale sits in e4m3 subnormals); 1/32 is folded into the q/k eviction
    # scales and, for the v path, into Wo/32 with bv*32 (the softmax
    # denominator is v-scale-invariant)
    hidden_states = np.ascontiguousarray(np.asarray(hidden_states, np.float32))
    bft = ml_dtypes.bfloat16
    Wq = np.asarray(Wq, np.float32) * np.float32(32.0)
    Wk = np.asarray(Wk, np.float32) * np.float32(32.0)
    Wv = np.asarray(Wv, np.float32) * np.float32(32.0)
    Wo = (np.asarray(Wo, np.float32) / np.float32(32.0)).astype(bft)
    bq = np.asarray(bq, np.float32)
    bk = np.asarray(bk, np.float32)
    bv = np.asarray(bv, np.float32) * np.float32(32.0)

    nearest, cb = _host_morpho(morpho_types)

    in_maps = []
    for c in range(8):
        b, g = c // G, c % G
        fs = slice(g * F, (g + 1) * F)
        x8, xr8 = _fp8_split(hidden_states[b].T)
        wq8, wqr8 = _fp8_split(Wq[:, fs])
        wk8, wkr8 = _fp8_split(Wk[:, fs])
        wv8, wvr8 = _fp8_split(Wv[:, fs])
        in_maps.append({
            "x": x8, "xr": xr8,
            "wq": wq8, "wqr": wqr8,
            "wk": wk8, "wkr": wkr8,
            "wv": wv8, "wvr": wvr8,
            "wo": np.ascontiguousarray(Wo[fs, :]),
            "bqs": np.ascontiguousarray(bq[fs]),
            "bk": np.ascontiguousarray(bk[fs]),
            "bv": np.ascontiguousarray(bv[fs]),
            "nearf": nearest[b],
            "cb": cb[b],
        })
    return in_maps


def kernel(hidden_states, morpho_types, Wq, bq, Wk, bk, Wv, bv, Wo, bo):
    bo = np.asarray(bo, np.float32)
    in_maps = build_in_maps(
        hidden_states, morpho_types, Wq, bq, Wk, bk, Wv, bv, Wo, bo
    )
    nc = _get_compiled()
    res = run_bass_kernel_spmd(nc, in_maps, core_ids=list(range(8)))
    out = np.empty((B, S, H), np.float32)
    for b in range(B):
        out[b] = (
            res.results[2 * b]["z"].astype(np.float32)
            + res.results[2 * b + 1]["z"].astype(np.float32)
            + bo
        )
    return out
